# revision 2
# baseline (speedup 1.0000x reference)
"""Trainium2 Bass kernel for nn_MultiHeadAttention (dense transformer, causal MHA).

Reference semantics (faithful to the PyTorch source bug):
    q/k/v = x @ Wq/Wk/Wv          -> [B, H, S, hd] (no bias)
    scores = q @ k^T, causal mask, softmax(scores / hd**2)
    ctx = weights @ v             -> [B, H, S, hd]
    ctx reshaped [B,H,S,hd] -> [B,S,D] WITHOUT head transpose-back
    out = ctx_reshaped @ Wo + bo

Sharding: 8 cores = 2 (batch) x 4 (head groups of 4 heads).  Because of the
buggy reshape, row block [128*h, 128*(h+1)) of the [S, D] ctx_reshaped matrix
comes entirely from head h, so each core independently computes full output
rows [512*hg, 512*(hg+1)) for its batch.  No collectives.

On-chip layout (all matmul operands float32r = fp32 data, single-pass PE mode):
    xT   [128, 8, 2048]   x[b] transposed (d on partitions), built via PE transpose
    qT/kT [128, 2, 2048]  per head-pair (partitions 0-63 head 2j, 64-127 head 2j+1)
    vaug [128, 16, 260]   v natural (seq on partitions) + ones column per head
    scores computed transposed: sT[n, m] = sum_e kT[e,n] qT[e,m]  (K=64)
    exp via ACT (scale=1/4096), causal mask via DVE affine_select on diagonal blocks
    ctx: PT[i, m] = sum_n vaug[n, i] expsT[n, m]  accumulated over n-blocks
         (i=0..63 ctx dims, i=64 = softmax denominator via the ones column)
    normalize via reciprocal + gpsimd partition_broadcast + DVE mul
    ctx2 [128, 1024]: ctx2[64*(m%2)+e, m//2] = ctx^T[e, m]  (via SBUF->SBUF DMA)
    out rows: lhsT chunk g = ctx2[:, g::8], rhs = Wo[128g:128(g+1), :]
"""

import numpy as np

B, S, D = 2, 2048, 1024
H, HD = 16, 64
N_CORES = 8
HPC = 4          # heads per core
ST_M = 256       # query supertile (free dim of scores matmuls)
N_ST = S // ST_M     # 8 supertiles
N_NB = S // 128      # 16 key blocks

_CACHE = {}


def _build():
    import concourse.bacc as bacc
    import concourse.mybir as mybir
    import concourse.tile as tile

    F32 = mybir.dt.float32
    F32R = mybir.dt.float32r
    EXP = mybir.ActivationFunctionType.Exp
    GE = mybir.AluOpType.is_ge

    nc = bacc.Bacc("TRN2", target_bir_lowering=False, debug=False,
                   num_devices=N_CORES)

    xb = nc.dram_tensor("xb", [S, D], F32, kind="ExternalInput").ap()
    wq = nc.dram_tensor("wq", [D, 256], F32, kind="ExternalInput").ap()
    wk = nc.dram_tensor("wk", [D, 256], F32, kind="ExternalInput").ap()
    wv = nc.dram_tensor("wv", [D, 256], F32, kind="ExternalInput").ap()
    wo = nc.dram_tensor("wo", [D, D], F32, kind="ExternalInput").ap()
    bo = nc.dram_tensor("bo", [D], F32, kind="ExternalInput").ap()
    ident = nc.dram_tensor("ident", [128, 128], F32, kind="ExternalInput").ap()
    ones = nc.dram_tensor("ones", [128, 64], F32, kind="ExternalInput").ap()
    outc = nc.dram_tensor("outc", [HPC * 128, D], F32, kind="ExternalOutput").ap()

    import concourse.bass as bass

    with tile.TileContext(nc) as tc:
        with tc.tile_pool(name="persist", bufs=1) as pp, \
             tc.tile_pool(name="qkvout", bufs=1) as qp:
            idt = pp.tile([128, 128], F32R)
            nc.sync.dma_start(out=idt, in_=ident.bitcast(F32R))
            onesb = pp.tile([128, 64], F32R)
            nc.sync.dma_start(out=onesb, in_=ones.bitcast(F32R))

            xT = pp.tile([128, 8, S], F32R)
            qT = qp.tile([128, 2, S], F32R, tag="qT")
            kT = qp.tile([128, 2, S], F32R, tag="kT")
            vaug = qp.tile([128, N_NB, 65 * HPC], F32R)

            # ---- phase A: build xT, then q/k/v projections ----
            with tc.tile_pool(name="wqkv", bufs=1) as wp, \
                 tc.tile_pool(name="xn", bufs=2) as xp, \
                 tc.tile_pool(name="ps_tr", bufs=2, space="PSUM") as ptr, \
                 tc.tile_pool(name="ps_qkv", bufs=2, space="PSUM") as pqk:
                wq_sb = wp.tile([128, 8, 256], F32R, tag="wq_sb")
                nc.sync.dma_start(
                    out=wq_sb,
                    in_=wq.bitcast(F32R).rearrange("(c p) n -> p c n", p=128))
                wk_sb = wp.tile([128, 8, 256], F32R, tag="wk_sb")
                nc.sync.dma_start(
                    out=wk_sb,
                    in_=wk.bitcast(F32R).rearrange("(c p) n -> p c n", p=128))
                wv_sb = wp.tile([128, 8, 256], F32R, tag="wv_sb")
                nc.sync.dma_start(
                    out=wv_sb,
                    in_=wv.bitcast(F32R).rearrange("(c p) n -> p c n", p=128))

                # x[b] transposed into xT via PE transpose (128x128 blocks)
                for st in range(16):
                    xn = xp.tile([128, D], F32R, tag="xn")
                    nc.sync.dma_start(
                        out=xn, in_=xb[128 * st:128 * (st + 1), :].bitcast(F32R))
                    for dg in range(2):
                        pt = ptr.tile([128, 512], F32R, tag="pt")
                        for d4 in range(4):
                            dc = 4 * dg + d4
                            nc.tensor.transpose(
                                pt[:, 128 * d4:128 * (d4 + 1)],
                                xn[:, 128 * dc:128 * (dc + 1)], idt)
                        nc.vector.tensor_copy(
                            out=xT[:, 4 * dg:4 * (dg + 1),
                                   128 * st:128 * (st + 1)],
                            in_=pt.rearrange("p (a b) -> p a b", b=128))

                # qT / kT: per head pair j, out [128, S]
                for w_sb, dstT in ((wq_sb, qT), (wk_sb, kT)):
                    for j in range(2):
                        for ck in range(4):
                            pq = pqk.tile([128, 512], F32, tag="pq")
                            for dc in range(8):
                                nc.tensor.matmul(
                                    pq,
                                    w_sb[:, dc, 128 * j:128 * (j + 1)],
                                    xT[:, dc, 512 * ck:512 * (ck + 1)],
                                    start=(dc == 0), stop=(dc == 7))
                            nc.vector.tensor_copy(
                                out=dstT[:, j, 512 * ck:512 * (ck + 1)], in_=pq)

                # v natural + ones columns
                for h in range(HPC):
                    nc.vector.tensor_copy(
                        out=vaug[:, :, 65 * h + 64:65 * h + 65],
                        in_=onesb[:, 0:N_NB].rearrange("p (a b) -> p a b", b=1))
                for nb in range(N_NB):
                    pv = pqk.tile([128, 256], F32, tag="pv")
                    for dc in range(8):
                        nc.tensor.matmul(
                            pv,
                            xT[:, dc, 128 * nb:128 * (nb + 1)],
                            wv_sb[:, dc, :],
                            start=(dc == 0), stop=(dc == 7))
                    for h in range(HPC):
                        nc.vector.tensor_copy(
                            out=vaug[:, nb, 65 * h:65 * h + 64],
                            in_=pv[:, 64 * h:64 * (h + 1)])

            # ---- phase B: attention + output projection, head by head ----
            with tc.tile_pool(name="wo_p", bufs=1) as wop, \
                 tc.tile_pool(name="sb_b", bufs=3) as sp, \
                 tc.tile_pool(name="ctx2_p", bufs=2) as cp, \
                 tc.tile_pool(name="ps_sc", bufs=3, space="PSUM") as psc, \
                 tc.tile_pool(name="ps_pt", bufs=2, space="PSUM") as ppt, \
                 tc.tile_pool(name="ps_out", bufs=2, space="PSUM") as pso:
                wo_sb = wop.tile([128, 8, D], F32R)
                nc.sync.dma_start(
                    out=wo_sb,
                    in_=wo.bitcast(F32R).rearrange("(c p) n -> p c n", p=128))
                bob = wop.tile([128, D], F32)
                nc.sync.dma_start(
                    out=bob,
                    in_=bass.AP(tensor=bo.tensor, offset=0,
                                ap=[[0, 128], [1, D]]))

                for h in range(HPC):
                    hp, ph = divmod(h, 2)
                    base = 64 * ph
                    ctx2 = cp.tile([128, S // 2], F32R, tag="ctx2")
                    for st in range(N_ST):
                        nlast = 2 * st + 1
                        PT = ppt.tile([65, ST_M], F32, tag="PT")
                        for nb in range(2 * st + 2):
                            SC = psc.tile([128, ST_M], F32, tag="SC")
                            nc.tensor.matmul(
                                SC,
                                kT[base:base + 64, hp,
                                   128 * nb:128 * (nb + 1)],
                                qT[base:base + 64, hp,
                                   ST_M * st:ST_M * (st + 1)],
                                start=True, stop=True)
                            E = sp.tile([128, ST_M], F32R, tag="E")
                            nc.scalar.activation(
                                out=E, in_=SC, func=EXP,
                                scale=1.0 / float(HD * HD))
                            if nb >= 2 * st:
                                EM = sp.tile([128, ST_M], F32R, tag="EM")
                                nc.gpsimd.affine_select(
                                    out=EM, in_=E, pattern=[[1, ST_M]],
                                    compare_op=GE, fill=0.0,
                                    base=ST_M * st - 128 * nb,
                                    channel_multiplier=-1)
                                Eu = EM
                            else:
                                Eu = E
                            nc.tensor.matmul(
                                PT, vaug[:, nb, 65 * h:65 * (h + 1)], Eu,
                                start=(nb == 0), stop=(nb == nlast))
                        rec = sp.tile([1, ST_M], F32R, tag="rec")
                        with nc.allow_low_precision(reason="f32r recip"):
                            nc.vector.reciprocal(rec, PT[64:65, :])
                        rbs = sp.tile([64, ST_M], F32R, tag="rbs")
                        nc.gpsimd.partition_broadcast(rbs, rec)
                        Sst = sp.tile([64, 2, ST_M // 2], F32R, tag="Sst")
                        nc.vector.tensor_mul(
                            Sst,
                            PT[0:64, :].rearrange("p (a two) -> p two a", two=2),
                            rbs.rearrange("p (a two) -> p two a", two=2))
                        for bp in range(2):
                            nc.sync.dma_start(
                                out=ctx2[64 * bp:64 * (bp + 1),
                                         128 * st:128 * (st + 1)],
                                in_=Sst[:, bp, :])
                    # output projection for this head
                    for hf in range(2):
                        PO = pso.tile([128, 512], F32, tag="PO")
                        for g in range(8):
                            nc.tensor.matmul(
                                PO, ctx2[:, g::8],
                                wo_sb[:, g, 512 * hf:512 * (hf + 1)],
                                start=(g == 0), stop=(g == 7))
                        Of = sp.tile([128, 512], F32, tag="Of")
                        nc.vector.tensor_add(
                            Of, PO, bob[:, 512 * hf:512 * (hf + 1)])
                        nc.sync.dma_start(
                            out=outc[128 * h:128 * (h + 1),
                                     512 * hf:512 * (hf + 1)],
                            in_=Of)

    nc.compile()
    return nc


def _get_nc():
    if "nc" not in _CACHE:
        _CACHE["nc"] = _build()
    return _CACHE["nc"]


def kernel(x, Wq, Wk, Wv, Wo, bo):
    from concourse.bass_utils import run_bass_kernel_spmd

    x = np.asarray(x, dtype=np.float32)
    Wq = np.asarray(Wq, dtype=np.float32)
    Wk = np.asarray(Wk, dtype=np.float32)
    Wv = np.asarray(Wv, dtype=np.float32)
    Wo = np.asarray(Wo, dtype=np.float32)
    bo = np.asarray(bo, dtype=np.float32)

    nc = _get_nc()
    ident = np.eye(128, dtype=np.float32)
    onesm = np.ones((128, 64), dtype=np.float32)
    in_maps = []
    for c in range(N_CORES):
        b, hg = divmod(c, HPC)
        cs = slice(256 * hg, 256 * (hg + 1))
        in_maps.append({
            "xb": np.ascontiguousarray(x[b]),
            "wq": np.ascontiguousarray(Wq[:, cs]),
            "wk": np.ascontiguousarray(Wk[:, cs]),
            "wv": np.ascontiguousarray(Wv[:, cs]),
            "wo": Wo,
            "bo": bo,
            "ident": ident,
            "ones": onesm,
        })
    res = run_bass_kernel_spmd(nc, in_maps, core_ids=list(range(N_CORES)))
    out = np.empty((B, S, D), dtype=np.float32)
    for c in range(N_CORES):
        b, hg = divmod(c, HPC)
        out[b, 512 * hg:512 * (hg + 1), :] = res.results[c]["outc"]
    return out


# revision 5
# speedup vs baseline: 1.1625x; 1.1625x over previous
"""Trainium2 Bass kernel for nn_MultiHeadAttention (dense transformer, causal MHA).

Reference semantics (faithful to the PyTorch source bug):
    q/k/v = x @ Wq/Wk/Wv          -> [B, H, S, hd] (no bias)
    scores = q @ k^T, causal mask, softmax(scores / hd**2)
    ctx = weights @ v             -> [B, H, S, hd]
    ctx reshaped [B,H,S,hd] -> [B,S,D] WITHOUT head transpose-back
    out = ctx_reshaped @ Wo + bo

Sharding: 8 cores = 2 (batch) x 4 (head groups of 4 heads).  Because of the
buggy reshape, row block [128*h, 128*(h+1)) of the [S, D] ctx_reshaped matrix
comes entirely from head h, so each core independently computes full output
rows [512*hg, 512*(hg+1)) for its batch.  No collectives.

All matmul operands are float32r (fp32 data, single-pass PE mode, ~tf32-level
operand rounding).  Scores are computed transposed (sT[n,m]) so the softmax
denominator comes from an appended ones-column in v (PE reduction along
partitions) and the ctx matmul needs no transposes anywhere.
"""

import numpy as np

B, S, D = 2, 2048, 1024
H, HD = 16, 64
N_CORES = 8
HPC = 4            # heads per core
ST_M = 512         # query supertile (free dim of scores matmuls)
N_ST = S // ST_M   # 4

_CACHE = {}


def _build():
    import concourse.bass as bass
    import concourse.bacc as bacc
    import concourse.mybir as mybir
    import concourse.tile as tile

    F32 = mybir.dt.float32
    F32R = mybir.dt.float32r
    EXP = mybir.ActivationFunctionType.Exp

    nc = bacc.Bacc("TRN2", target_bir_lowering=False, debug=False,
                   num_devices=N_CORES)

    xb = nc.dram_tensor("xb", [S, D], F32, kind="ExternalInput").ap()
    wq = nc.dram_tensor("wq", [D, 256], F32, kind="ExternalInput").ap()
    wk = nc.dram_tensor("wk", [D, 256], F32, kind="ExternalInput").ap()
    wv = nc.dram_tensor("wv", [D, 256], F32, kind="ExternalInput").ap()
    wo = nc.dram_tensor("wo", [D, D], F32, kind="ExternalInput").ap()
    bo = nc.dram_tensor("bo", [D], F32, kind="ExternalInput").ap()
    ident = nc.dram_tensor("ident", [128, 128], F32, kind="ExternalInput").ap()
    ones = nc.dram_tensor("ones", [128, 64], F32, kind="ExternalInput").ap()
    # maskc[:, 0:128] = 0, maskc[:, 128:256] = T where T[p, j] = (j >= p)
    maskc = nc.dram_tensor("maskc", [128, 256], F32, kind="ExternalInput").ap()
    outc = nc.dram_tensor("outc", [HPC * 128, D], F32, kind="ExternalOutput").ap()

    with tile.TileContext(nc) as tc:
        with tc.tile_pool(name="persist", bufs=1) as pp, \
             tc.tile_pool(name="qkvout", bufs=1) as qp:
            idt = pp.tile([128, 128], F32R)
            nc.sync.dma_start(out=idt, in_=ident.bitcast(F32R))

            xT = pp.tile([128, 8, S], F32R)
            qT = qp.tile([128, 2, S], F32R, tag="qT")
            kT = qp.tile([128, 2, S], F32R, tag="kT")
            vaug = qp.tile([128, 16, 65 * HPC], F32R)
            onesb = pp.tile([128, 64], F32R)
            maskb = pp.tile([128, 256], F32R)
            wo_sb = pp.tile([128, 8, D], F32R)
            bob = pp.tile([128, D], F32)

            # ---- phase A: build xT, then q/k/v projections ----
            with tc.tile_pool(name="wqkv", bufs=1) as wp, \
                 tc.tile_pool(name="xn", bufs=2) as xp, \
                 tc.tile_pool(name="ps_tr", bufs=2, space="PSUM") as ptr, \
                 tc.tile_pool(name="ps_qkv", bufs=2, space="PSUM") as pqk:
                # x[b] transposed into xT via PE transpose (128x128 blocks)
                for st in range(16):
                    xn = xp.tile([128, D], F32R, tag="xn")
                    nc.sync.dma_start(
                        out=xn, in_=xb[128 * st:128 * (st + 1), :].bitcast(F32R))
                    for dg in range(2):
                        pt = ptr.tile([128, 512], F32R, tag="pt")
                        for d4 in range(4):
                            dc = 4 * dg + d4
                            nc.tensor.transpose(
                                pt[:, 128 * d4:128 * (d4 + 1)],
                                xn[:, 128 * dc:128 * (dc + 1)], idt)
                        nc.vector.tensor_copy(
                            out=xT[:, 4 * dg:4 * (dg + 1),
                                   128 * st:128 * (st + 1)],
                            in_=pt.rearrange("p (a b) -> p a b", b=128))

                wq_sb = wp.tile([128, 8, 256], F32R, tag="wq_sb")
                nc.sync.dma_start(
                    out=wq_sb,
                    in_=wq.bitcast(F32R).rearrange("(c p) n -> p c n", p=128))
                wk_sb = wp.tile([128, 8, 256], F32R, tag="wk_sb")
                nc.sync.dma_start(
                    out=wk_sb,
                    in_=wk.bitcast(F32R).rearrange("(c p) n -> p c n", p=128))
                wv_sb = wp.tile([128, 8, 256], F32R, tag="wv_sb")
                nc.sync.dma_start(
                    out=wv_sb,
                    in_=wv.bitcast(F32R).rearrange("(c p) n -> p c n", p=128))
                nc.sync.dma_start(out=onesb, in_=ones.bitcast(F32R))
                nc.sync.dma_start(out=maskb, in_=maskc.bitcast(F32R))

                # qT / kT: per head pair j, out [128, S]
                for w_sb, dstT in ((wq_sb, qT), (wk_sb, kT)):
                    for j in range(2):
                        for ck in range(4):
                            pq = pqk.tile([128, 512], F32, tag="pq")
                            for dc in range(8):
                                nc.tensor.matmul(
                                    pq,
                                    w_sb[:, dc, 128 * j:128 * (j + 1)],
                                    xT[:, dc, 512 * ck:512 * (ck + 1)],
                                    start=(dc == 0), stop=(dc == 7))
                            nc.vector.tensor_copy(
                                out=dstT[:, j, 512 * ck:512 * (ck + 1)], in_=pq)

                # v natural + ones columns
                for h in range(HPC):
                    nc.vector.tensor_copy(
                        out=vaug[:, :, 65 * h + 64:65 * h + 65],
                        in_=onesb[:, 0:16].rearrange("p (a b) -> p a b", b=1))
                for nb in range(16):
                    pv = pqk.tile([128, 256], F32, tag="pv")
                    for dc in range(8):
                        nc.tensor.matmul(
                            pv,
                            xT[:, dc, 128 * nb:128 * (nb + 1)],
                            wv_sb[:, dc, :],
                            start=(dc == 0), stop=(dc == 7))
                    for h in range(HPC):
                        nc.vector.tensor_copy(
                            out=vaug[:, nb, 65 * h:65 * h + 64],
                            in_=pv[:, 64 * h:64 * (h + 1)])

                # Wo / bias load late in phase A (needed only in phase B)
                nc.sync.dma_start(
                    out=wo_sb,
                    in_=wo.bitcast(F32R).rearrange("(c p) n -> p c n", p=128))
                nc.sync.dma_start(
                    out=bob,
                    in_=bass.AP(tensor=bo.tensor, offset=0,
                                ap=[[0, 128], [1, D]]))

            # ---- phase B: attention + output projection, pair-interleaved ----
            with tc.tile_pool(name="sb_b", bufs=3) as sp, \
                 tc.tile_pool(name="sb_c", bufs=2) as cp2, \
                 tc.tile_pool(name="ps_sc", bufs=3, space="PSUM") as psc, \
                 tc.tile_pool(name="ps_pt", bufs=2, space="PSUM") as ppt, \
                 tc.tile_pool(name="ps_out", bufs=1, space="PSUM") as pso:
                for hp in range(2):
                    ctx2 = [cp2.tile([128, S // 2], F32R, tag=f"ctx2_{ph}",
                                     name=f"ctx2_{ph}") for ph in range(2)]
                    for st in range(N_ST):
                        nlast = 4 * st + 3
                        PT = [ppt.tile([65, ST_M], F32, tag=f"PT{ph}",
                                       name=f"PT{ph}") for ph in range(2)]
                        for nb in range(4 * st + 4):
                            d = nb - 4 * st   # >= 0 on diagonal blocks
                            # valid query sub-range of this supertile
                            lo = 0 if d < 0 else (256 if d == 3 else 128 * d)
                            w = ST_M - lo
                            for ph in range(2):
                                h = 2 * hp + ph
                                base = 64 * ph
                                SC = psc.tile([128, ST_M], F32, tag="SC")
                                nc.tensor.matmul(
                                    SC[:, lo:],
                                    kT[base:base + 64, hp,
                                       128 * nb:128 * (nb + 1)],
                                    qT[base:base + 64, hp,
                                       ST_M * st + lo:ST_M * (st + 1)],
                                    start=True, stop=True)
                                E = sp.tile([128, ST_M], F32R, tag="E")
                                nc.scalar.activation(
                                    out=E[:, lo:], in_=SC[:, lo:], func=EXP,
                                    scale=1.0 / float(HD * HD))
                                if d == 3:
                                    nc.vector.tensor_mul(
                                        E[:, lo:], E[:, lo:], maskb)
                                elif d >= 0:
                                    nc.vector.tensor_mul(
                                        E[:, lo:lo + 128], E[:, lo:lo + 128],
                                        maskb[:, 128:256])
                                nc.tensor.matmul(
                                    PT[ph][:, lo:],
                                    vaug[:, nb, 65 * h:65 * (h + 1)],
                                    E[:, lo:],
                                    start=(nb == 0), stop=(nb == nlast))
                        for ph in range(2):
                            rec = sp.tile([1, ST_M], F32, tag="rec")
                            with nc.allow_low_precision(reason="recip"):
                                nc.vector.reciprocal(
                                    out=rec, in_=PT[ph][64:65, :])
                            rbs = sp.tile([64, ST_M], F32, tag="rbs")
                            nc.gpsimd.partition_broadcast(rbs, rec)
                            Sst = sp.tile([64, 2, ST_M // 2], F32R, tag="Sst")
                            nc.vector.tensor_mul(
                                Sst,
                                PT[ph][0:64, :].rearrange(
                                    "p (a two) -> p two a", two=2),
                                rbs.rearrange("p (a two) -> p two a", two=2))
                            for bp in range(2):
                                nc.sync.dma_start(
                                    out=ctx2[ph][64 * bp:64 * (bp + 1),
                                                 (ST_M // 2) * st:
                                                 (ST_M // 2) * (st + 1)],
                                    in_=Sst[:, bp, :])
                    # output projection for both heads of the pair
                    for ph in range(2):
                        h = 2 * hp + ph
                        for hf in range(2):
                            PO = pso.tile([128, 512], F32, tag="PO")
                            for g in range(8):
                                nc.tensor.matmul(
                                    PO, ctx2[ph][:, g::8],
                                    wo_sb[:, g, 512 * hf:512 * (hf + 1)],
                                    start=(g == 0), stop=(g == 7))
                            Of = sp.tile([128, 512], F32, tag="Of")
                            nc.vector.tensor_add(
                                Of, PO, bob[:, 512 * hf:512 * (hf + 1)])
                            nc.sync.dma_start(
                                out=outc[128 * h:128 * (h + 1),
                                         512 * hf:512 * (hf + 1)],
                                in_=Of)

    nc.compile()
    return nc


def _get_nc():
    if "nc" not in _CACHE:
        _CACHE["nc"] = _build()
    return _CACHE["nc"]


def kernel(x, Wq, Wk, Wv, Wo, bo):
    from concourse.bass_utils import run_bass_kernel_spmd

    x = np.asarray(x, dtype=np.float32)
    Wq = np.asarray(Wq, dtype=np.float32)
    Wk = np.asarray(Wk, dtype=np.float32)
    Wv = np.asarray(Wv, dtype=np.float32)
    Wo = np.asarray(Wo, dtype=np.float32)
    bo = np.asarray(bo, dtype=np.float32)

    nc = _get_nc()
    ident = np.eye(128, dtype=np.float32)
    onesm = np.ones((128, 64), dtype=np.float32)
    tri = (np.arange(128)[None, :] >= np.arange(128)[:, None])
    maskc = np.concatenate(
        [np.zeros((128, 128), np.float32), tri.astype(np.float32)], axis=1)
    in_maps = []
    for c in range(N_CORES):
        b, hg = divmod(c, HPC)
        cs = slice(256 * hg, 256 * (hg + 1))
        in_maps.append({
            "xb": np.ascontiguousarray(x[b]),
            "wq": np.ascontiguousarray(Wq[:, cs]),
            "wk": np.ascontiguousarray(Wk[:, cs]),
            "wv": np.ascontiguousarray(Wv[:, cs]),
            "wo": Wo,
            "bo": bo,
            "ident": ident,
            "ones": onesm,
            "maskc": maskc,
        })
    res = run_bass_kernel_spmd(nc, in_maps, core_ids=list(range(N_CORES)))
    out = np.empty((B, S, D), dtype=np.float32)
    for c in range(N_CORES):
        b, hg = divmod(c, HPC)
        out[b, 512 * hg:512 * (hg + 1), :] = res.results[c]["outc"]
    return out
